# revision 17
# baseline (speedup 1.0000x reference)
"""MoE routing kernel for Trainium2 (8 NeuronCores, SPMD).

Math being implemented (faithful to the reference, including its quirks):
  logits = x @ gate_w + gate_b                  # [B,S,E]
  weights = softmax(logits, axis=1)             # softmax over the SEQUENCE axis
  top2 values/indices over experts; only experts 0 and 1 are ever evaluated
  (the reference loops `for ind in range(top_k)` and uses expert `ind`).
  out[t] = c0[t]*eo_0[t] + c1[t]*eo_1[t], where
  eo_e = softmax_D(gelu(x@w1[e]+b1[e]) @ w2[e] + b2[e]) and c_e[t] is the
  top-2 gate weight when expert e is in token t's top-2, else 0.

Sharding strategy: routing + dispatch on host (0.4% of FLOPs). Only tokens
whose top-2 contains expert 0/1 are computed (~25% each). Cores 0-3 handle
expert 0's tokens, cores 4-7 expert 1's, so each core streams only one
expert's weights. The FFN+softmax runs on-device in feature-major layout.
"""

import sys

import numpy as np

sys.path.insert(0, "/opt/trn_rl_repo")

import concourse.bacc as bacc  # noqa: E402
import concourse.bass as bass  # noqa: E402
import concourse.tile as tile  # noqa: E402
from concourse import mybir  # noqa: E402
from concourse.bass_utils import run_bass_kernel_spmd  # noqa: E402

P = 128
D = 1024
F = 4096
NCORES = 8
CHUNK = 512  # max matmul moving free dim (fp32/f32r)
AF = mybir.ActivationFunctionType

_CACHE = {}


def _gating_coeffs(x, gate_w, gate_b):
    """Host replica of the reference gating. Returns c[T,2] float32 where
    c[:,e] is the gate weight if expert e is in the token's top-2 else 0."""
    B, S, _ = x.shape
    x = np.asarray(x, dtype=np.float32)
    logits = x.reshape(B * S, -1) @ np.asarray(gate_w, dtype=np.float32)
    logits = logits.reshape(B, S, -1) + np.asarray(gate_b, dtype=np.float32)
    # softmax over the sequence axis (axis=1), as in the reference
    m = logits.max(axis=1, keepdims=True)
    e = np.exp(logits - m)
    w = e / e.sum(axis=1, keepdims=True)
    wf = w.reshape(B * S, -1)
    # stable argsort of -w == jax.lax.top_k tie semantics (lower index wins)
    top2 = np.argsort(-wf, axis=-1, kind="stable")[:, :2]
    c = np.zeros((B * S, 2), dtype=np.float32)
    for ex in (0, 1):
        sel = (top2 == ex).any(axis=1)
        c[sel, ex] = wf[sel, ex]
    return c


def _round_f32r(a):
    """Round fp32 to the FP32R format (e8m11: RNE to 11 mantissa bits,
    low 12 bits zero), matching walrus' fp32_to_fp32r."""
    u = np.ascontiguousarray(a, dtype=np.float32).view(np.uint32)
    lsb = (u >> 12) & 1
    u = (u + 0x7FF + lsb) & np.uint32(0xFFFFF000)
    return u.view(np.float32)


def _build_nc(n, use_bf16):
    """Bass program for one core: n tokens (multiple of 128), one expert.

    Feature-major layout throughout: activations are [feature_tile(128), token].
      h^T = gelu(w1^T x^T + b1);  z^T = w2^T h^T + b2;  p = exp(z^T)
      s = colsum_D(p) (ones-matmul);  g = c / s (lane-parallel via DRAM bounce)
      out^T = p * broadcast(g)
    """
    dt = mybir.dt
    sdt = dt.bfloat16 if use_bf16 else dt.float32r  # storage for x/h/w
    f32 = dt.float32
    nchunks = (n + CHUNK - 1) // CHUNK
    chunks = []
    off = 0
    while off < n:
        sz = min(CHUNK, n - off)
        chunks.append((off, sz))
        off += sz
    KD, KF = D // P, F // P  # 8, 32
    # psum-group width (output tiles accumulated concurrently): keep
    # group_size * nchunks <= 4 PSUM banks
    mga = max(1, 4 // nchunks)

    nc = bacc.Bacc()
    xT = nc.dram_tensor("xT", [D, n], sdt, kind="ExternalInput")
    w1d = nc.dram_tensor("w1", [D, F], sdt, kind="ExternalInput")
    w2d = nc.dram_tensor("w2", [F, D], sdt, kind="ExternalInput")
    b1d = nc.dram_tensor("b1t", [P, KF], f32, kind="ExternalInput")
    b2d = nc.dram_tensor("b2t", [P, KD], f32, kind="ExternalInput")
    cd = nc.dram_tensor("clanes", [P, n // P], f32, kind="ExternalInput")
    outT = nc.dram_tensor("outT", [D, n], f32, kind="ExternalOutput")
    sdr = nc.dram_tensor("sdr", [n], f32)
    gdr = nc.dram_tensor("gdr", [n], f32)

    def asmm(ap):  # matmul operands are natively f32r/bf16-typed
        return ap

    with tile.TileContext(nc) as tc:
        with (
            tc.tile_pool(name="const", bufs=1) as const,
            tc.tile_pool(name="acts", bufs=1) as acts,
            tc.tile_pool(name="wpool", bufs=8) as wpool,
            tc.tile_pool(name="gp", bufs=1) as gp,
        ):
            b1t = const.tile([P, KF], f32)
            nc.sync.dma_start(b1t[:], b1d[:])
            b2t = const.tile([P, KD], f32)
            nc.sync.dma_start(b2t[:], b2d[:])
            clanes = const.tile([P, n // P], f32)
            nc.sync.dma_start(clanes[:], cd[:])
            ones_col = const.tile([P, 1], f32)
            nc.vector.memset(ones_col[:], 1.0)
            ones_row = const.tile([1, P], f32)
            nc.vector.memset(ones_row[:], 1.0)

            xs = acts.tile([P, KD * n], sdt)
            for k in range(KD):
                nc.sync.dma_start(
                    xs[:, k * n : (k + 1) * n], xT[k * P : (k + 1) * P, :]
                )
            h = acts.tile([P, KF * n], sdt)
            p = acts.tile([P, KD * n], f32)

            ab_pools = tc.tile_pool(name="psa", bufs=4, space="PSUM")
            psa_pool = ab_pools.__enter__()
            ab_pools2 = tc.tile_pool(name="psb", bufs=4, space="PSUM")
            psb_pool = ab_pools2.__enter__()

            # ---- Phase A: h = gelu(w1.T @ x.T + b1) ----
            for mg0 in range(0, KF, mga):
                msz = min(mga, KF - mg0)
                psas = {}
                for mi in range(msz):
                    for ci in range(nchunks):
                        psas[(mi, ci)] = psa_pool.tile(
                            [P, chunks[ci][1]], f32, tag="psa", name=f"psa_{mg0}_{mi}_{ci}"
                        )
                for k in range(KD):
                    wslab = wpool.tile([P, msz * P], sdt, tag="ws", name=f"w1s_{mg0}_{k}")
                    nc.gpsimd.dma_start(
                        wslab[:], w1d[k * P : (k + 1) * P, mg0 * P : (mg0 + msz) * P]
                    )
                    for mi in range(msz):
                        for ci, (c0, csz) in enumerate(chunks):
                            nc.tensor.matmul(
                                psas[(mi, ci)][:],
                                asmm(wslab[:, mi * P : (mi + 1) * P]),
                                asmm(xs[:, k * n + c0 : k * n + c0 + csz]),
                                start=(k == 0),
                                stop=(k == KD - 1),
                            )
                for mi in range(msz):
                    m = mg0 + mi
                    for ci, (c0, csz) in enumerate(chunks):
                        nc.scalar.activation(
                            h[:, m * n + c0 : m * n + c0 + csz],
                            psas[(mi, ci)][:],
                            AF.Gelu,
                            bias=b1t[:, m : m + 1],
                        )

            # ---- Phase B: p = exp(w2.T @ h + b2) ----
            for mg0 in range(0, KD, mga):
                msz = min(mga, KD - mg0)
                psbs = {}
                for mi in range(msz):
                    for ci in range(nchunks):
                        psbs[(mi, ci)] = psb_pool.tile(
                            [P, chunks[ci][1]], f32, tag="psb", name=f"psb_{mg0}_{mi}_{ci}"
                        )
                for k in range(KF):
                    wslab = wpool.tile([P, msz * P], sdt, tag="ws", name=f"w2s_{mg0}_{k}")
                    nc.gpsimd.dma_start(
                        wslab[:], w2d[k * P : (k + 1) * P, mg0 * P : (mg0 + msz) * P]
                    )
                    for mi in range(msz):
                        for ci, (c0, csz) in enumerate(chunks):
                            nc.tensor.matmul(
                                psbs[(mi, ci)][:],
                                asmm(wslab[:, mi * P : (mi + 1) * P]),
                                asmm(h[:, k * n + c0 : k * n + c0 + csz]),
                                start=(k == 0),
                                stop=(k == KF - 1),
                            )
                for mi in range(msz):
                    m = mg0 + mi
                    for ci, (c0, csz) in enumerate(chunks):
                        nc.scalar.activation(
                            p[:, m * n + c0 : m * n + c0 + csz],
                            psbs[(mi, ci)][:],
                            AF.Exp,
                            bias=b2t[:, m : m + 1],
                        )

            ab_pools2.__exit__(None, None, None)
            ab_pools.__exit__(None, None, None)
            pss_ctx = tc.tile_pool(name="pss", bufs=1, space="PSUM")
            pss_pool = pss_ctx.__enter__()

            # ---- Phase C: normalize and scale by gate coefficient ----
            s_sb = gp.tile([1, n], f32)
            for ci, (c0, csz) in enumerate(chunks):
                sp = pss_pool.tile([1, csz], f32, tag="sp", name=f"sp_{ci}")
                for k in range(KD):
                    nc.tensor.matmul(
                        sp[:],
                        ones_col[:],
                        p[:, k * n + c0 : k * n + c0 + csz],
                        start=(k == 0),
                        stop=(k == KD - 1),
                    )
                nc.vector.tensor_copy(s_sb[0:1, c0 : c0 + csz], sp[:])
            # lane-parallel g = c / s via DRAM round-trip reshape
            nc.sync.dma_start(sdr[:].rearrange("(o f) -> o f", o=1), s_sb[0:1, :])
            slanes = gp.tile([P, n // P], f32)
            nc.sync.dma_start(slanes[:], sdr[:].rearrange("(p f) -> p f", p=P))
            rlanes = gp.tile([P, n // P], f32)
            nc.vector.reciprocal(rlanes[:], slanes[:])
            glanes = gp.tile([P, n // P], f32)
            nc.vector.tensor_mul(glanes[:], rlanes[:], clanes[:])
            nc.sync.dma_start(gdr[:].rearrange("(p f) -> p f", p=P), glanes[:])
            g_sb = gp.tile([1, n], f32)
            nc.sync.dma_start(g_sb[:], gdr[:].rearrange("(o f) -> o f", o=1))
            for ci, (c0, csz) in enumerate(chunks):
                gb_ps = pss_pool.tile([P, csz], f32, tag="gb", name=f"gb_{ci}")
                nc.tensor.matmul(
                    gb_ps[:],
                    ones_row[:],
                    g_sb[0:1, c0 : c0 + csz],
                    start=True,
                    stop=True,
                )
                gb = gp.tile([P, csz], f32, tag="gbs", name=f"gbs_{ci}")
                nc.vector.tensor_copy(gb[:], gb_ps[:])
                for k in range(KD):
                    nc.vector.tensor_mul(
                        p[:, k * n + c0 : k * n + c0 + csz],
                        p[:, k * n + c0 : k * n + c0 + csz],
                        gb[:],
                    )
            for k in range(KD):
                nc.sync.dma_start(outT[k * P : (k + 1) * P, :], p[:, k * n : (k + 1) * n])
            pss_ctx.__exit__(None, None, None)

    nc.finalize()
    return nc


def _get_nc(n, use_bf16):
    key = (n, use_bf16)
    if key not in _CACHE:
        _CACHE[key] = _build_nc(n, use_bf16)
    return _CACHE[key]


def kernel(x, gate_w, gate_b, w1, b1, w2, b2, top_k, use_bf16=False):
    x = np.asarray(x)
    B, S, _ = x.shape
    T = B * S
    assert int(top_k) == 2
    c = _gating_coeffs(x, gate_w, gate_b)

    x_f = np.ascontiguousarray(x.reshape(T, D).astype(np.float32))
    idx = [np.nonzero(c[:, ex])[0] for ex in (0, 1)]  # tokens per expert
    per_core = max(
        (len(idx[0]) + 3) // 4, (len(idx[1]) + 3) // 4, 1
    )
    n = ((per_core + P - 1) // P) * P  # padded tokens per core

    if use_bf16:
        import ml_dtypes

        def conv(a):
            return np.ascontiguousarray(a.astype(ml_dtypes.bfloat16))
    else:
        conv = _round_f32r

    w1 = np.asarray(w1, dtype=np.float32)
    w2 = np.asarray(w2, dtype=np.float32)
    b1 = np.asarray(b1, dtype=np.float32)
    b2 = np.asarray(b2, dtype=np.float32)

    in_maps = []
    core_tok = []  # per-core real token ids
    for core in range(NCORES):
        ex = core // 4
        part = core % 4
        ids = idx[ex][part * per_core : (part + 1) * per_core]
        core_tok.append(ids)
        xTc = np.zeros((D, n), dtype=np.float32)
        if len(ids):
            xTc[:, : len(ids)] = x_f[ids].T
        cl = np.zeros(n, dtype=np.float32)
        cl[: len(ids)] = c[ids, ex]
        in_maps.append(
            {
                "xT": conv(xTc),
                "w1": conv(w1[ex]),
                "w2": conv(w2[ex]),
                "b1t": np.ascontiguousarray(b1[ex].reshape(F // P, P).T.astype(np.float32)),
                "b2t": np.ascontiguousarray(b2[ex].reshape(D // P, P).T.astype(np.float32)),
                "clanes": np.ascontiguousarray(cl.reshape(P, n // P)),
            }
        )

    nc = _get_nc(n, use_bf16)
    res = run_bass_kernel_spmd(nc, in_maps, core_ids=list(range(NCORES)))
    kernel.last_results = res

    out = np.zeros((T, D), dtype=np.float32)
    for core in range(NCORES):
        ids = core_tok[core]
        if len(ids) == 0:
            continue
        contrib = res.results[core]["outT"][:, : len(ids)].T  # [n_real, D]
        out[ids] += contrib
    return out.reshape(B, S, D)


kernel.last_results = None


# revision 19
# speedup vs baseline: 69.7044x; 69.7044x over previous
"""MoE routing kernel for Trainium2 (8 NeuronCores, SPMD).

Math being implemented (faithful to the reference, including its quirks):
  logits = x @ gate_w + gate_b                  # [B,S,E]
  weights = softmax(logits, axis=1)             # softmax over the SEQUENCE axis
  top2 values/indices over experts; only experts 0 and 1 are ever evaluated
  (the reference loops `for ind in range(top_k)` and uses expert `ind`).
  out[t] = c0[t]*eo_0[t] + c1[t]*eo_1[t], where
  eo_e = softmax_D(gelu(x@w1[e]+b1[e]) @ w2[e] + b2[e]) and c_e[t] is the
  top-2 gate weight when expert e is in token t's top-2, else 0.

Sharding strategy: routing + dispatch on host (0.4% of FLOPs). Only tokens
whose top-2 contains expert 0/1 are computed (~25% each). Cores 0-3 handle
expert 0's tokens, cores 4-7 expert 1's, so each core streams only one
expert's weights. The FFN+softmax runs on-device in feature-major layout.
"""

import sys

import numpy as np

sys.path.insert(0, "/opt/trn_rl_repo")

import concourse.bacc as bacc  # noqa: E402
import concourse.bass as bass  # noqa: E402
import concourse.tile as tile  # noqa: E402
from concourse import mybir  # noqa: E402
from concourse.bass_utils import run_bass_kernel_spmd  # noqa: E402

P = 128
D = 1024
F = 4096
NCORES = 8
CHUNK = 512  # max matmul moving free dim (fp32/f32r)
AF = mybir.ActivationFunctionType

_CACHE = {}


def _gating_coeffs(x, gate_w, gate_b):
    """Host replica of the reference gating. Returns c[T,2] float32 where
    c[:,e] is the gate weight if expert e is in the token's top-2 else 0."""
    B, S, _ = x.shape
    x = np.asarray(x, dtype=np.float32)
    logits = x.reshape(B * S, -1) @ np.asarray(gate_w, dtype=np.float32)
    logits = logits.reshape(B, S, -1) + np.asarray(gate_b, dtype=np.float32)
    # softmax over the sequence axis (axis=1), as in the reference
    m = logits.max(axis=1, keepdims=True)
    e = np.exp(logits - m)
    w = e / e.sum(axis=1, keepdims=True)
    wf = w.reshape(B * S, -1)
    # stable argsort of -w == jax.lax.top_k tie semantics (lower index wins)
    top2 = np.argsort(-wf, axis=-1, kind="stable")[:, :2]
    c = np.zeros((B * S, 2), dtype=np.float32)
    for ex in (0, 1):
        sel = (top2 == ex).any(axis=1)
        c[sel, ex] = wf[sel, ex]
    return c


def _round_f32r(a):
    """Round fp32 to the FP32R format (e8m11: RNE to 11 mantissa bits,
    low 12 bits zero), matching walrus' fp32_to_fp32r."""
    u = np.ascontiguousarray(a, dtype=np.float32).view(np.uint32)
    lsb = (u >> 12) & 1
    u = (u + 0x7FF + lsb) & np.uint32(0xFFFFF000)
    return u.view(np.float32)


def _build_nc(n, use_bf16):
    """Bass program for one core: n tokens (multiple of 128), one expert.

    Feature-major layout throughout: activations are [feature_tile(128), token].
      h^T = gelu(w1^T x^T + b1);  z^T = w2^T h^T + b2;  p = exp(z^T)
      s = colsum_D(p) (ones-matmul);  g = c / s (lane-parallel via DRAM bounce)
      out^T = p * broadcast(g)
    """
    dt = mybir.dt
    sdt = dt.bfloat16 if use_bf16 else dt.float32r  # storage for x/h/w
    f32 = dt.float32
    nchunks = (n + CHUNK - 1) // CHUNK
    chunks = []
    off = 0
    while off < n:
        sz = min(CHUNK, n - off)
        chunks.append((off, sz))
        off += sz
    KD, KF = D // P, F // P  # 8, 32
    # psum-group width (output tiles accumulated concurrently): keep
    # group_size * nchunks <= 4 PSUM banks
    mga = max(1, 4 // nchunks)

    nc = bacc.Bacc()
    xT = nc.dram_tensor("xT", [D, n], sdt, kind="ExternalInput")
    w1d = nc.dram_tensor("w1", [D, F], sdt, kind="ExternalInput")
    w2d = nc.dram_tensor("w2", [F, D], sdt, kind="ExternalInput")
    b1d = nc.dram_tensor("b1t", [P, KF], f32, kind="ExternalInput")
    b2d = nc.dram_tensor("b2t", [P, KD], f32, kind="ExternalInput")
    cd = nc.dram_tensor("clanes", [P, n // P], f32, kind="ExternalInput")
    outT = nc.dram_tensor("outT", [D, n], f32, kind="ExternalOutput")
    sdr = nc.dram_tensor("sdr", [n], f32)
    gdr = nc.dram_tensor("gdr", [n], f32)

    def asmm(ap):  # matmul operands are natively f32r/bf16-typed
        return ap

    with tile.TileContext(nc) as tc:
        with (
            tc.tile_pool(name="const", bufs=1) as const,
            tc.tile_pool(name="acts", bufs=1) as acts,
            tc.tile_pool(name="wpool", bufs=8) as wpool,
            tc.tile_pool(name="gp", bufs=1) as gp,
        ):
            b1t = const.tile([P, KF], f32)
            nc.sync.dma_start(b1t[:], b1d[:])
            b2t = const.tile([P, KD], f32)
            nc.sync.dma_start(b2t[:], b2d[:])
            clanes = const.tile([P, n // P], f32)
            nc.sync.dma_start(clanes[:], cd[:])
            ones_col = const.tile([P, 1], f32)
            nc.vector.memset(ones_col[:], 1.0)
            ones_row = const.tile([1, P], f32)
            nc.vector.memset(ones_row[:], 1.0)

            xs = acts.tile([P, KD * n], sdt)
            for k in range(KD):
                nc.sync.dma_start(
                    xs[:, k * n : (k + 1) * n], xT[k * P : (k + 1) * P, :]
                )
            h = acts.tile([P, KF * n], sdt)
            p = acts.tile([P, KD * n], f32)

            ab_pools = tc.tile_pool(name="psa", bufs=4, space="PSUM")
            psa_pool = ab_pools.__enter__()
            ab_pools2 = tc.tile_pool(name="psb", bufs=4, space="PSUM")
            psb_pool = ab_pools2.__enter__()

            # ---- Phase A: h = gelu(w1.T @ x.T + b1) ----
            for mg0 in range(0, KF, mga):
                msz = min(mga, KF - mg0)
                psas = {}
                for mi in range(msz):
                    for ci in range(nchunks):
                        psas[(mi, ci)] = psa_pool.tile(
                            [P, chunks[ci][1]], f32, tag="psa", name=f"psa_{mg0}_{mi}_{ci}"
                        )
                for k in range(KD):
                    wslab = wpool.tile([P, msz * P], sdt, tag="ws", name=f"w1s_{mg0}_{k}")
                    nc.gpsimd.dma_start(
                        wslab[:], w1d[k * P : (k + 1) * P, mg0 * P : (mg0 + msz) * P]
                    )
                    for mi in range(msz):
                        for ci, (c0, csz) in enumerate(chunks):
                            nc.tensor.matmul(
                                psas[(mi, ci)][:],
                                asmm(wslab[:, mi * P : (mi + 1) * P]),
                                asmm(xs[:, k * n + c0 : k * n + c0 + csz]),
                                start=(k == 0),
                                stop=(k == KD - 1),
                            )
                for mi in range(msz):
                    m = mg0 + mi
                    for ci, (c0, csz) in enumerate(chunks):
                        nc.scalar.activation(
                            h[:, m * n + c0 : m * n + c0 + csz],
                            psas[(mi, ci)][:],
                            AF.Gelu,
                            bias=b1t[:, m : m + 1],
                        )

            # ---- Phase B: p = exp(w2.T @ h + b2) ----
            for mg0 in range(0, KD, mga):
                msz = min(mga, KD - mg0)
                psbs = {}
                for mi in range(msz):
                    for ci in range(nchunks):
                        psbs[(mi, ci)] = psb_pool.tile(
                            [P, chunks[ci][1]], f32, tag="psb", name=f"psb_{mg0}_{mi}_{ci}"
                        )
                for k in range(KF):
                    wslab = wpool.tile([P, msz * P], sdt, tag="ws", name=f"w2s_{mg0}_{k}")
                    nc.gpsimd.dma_start(
                        wslab[:], w2d[k * P : (k + 1) * P, mg0 * P : (mg0 + msz) * P]
                    )
                    for mi in range(msz):
                        for ci, (c0, csz) in enumerate(chunks):
                            nc.tensor.matmul(
                                psbs[(mi, ci)][:],
                                asmm(wslab[:, mi * P : (mi + 1) * P]),
                                asmm(h[:, k * n + c0 : k * n + c0 + csz]),
                                start=(k == 0),
                                stop=(k == KF - 1),
                            )
                for mi in range(msz):
                    m = mg0 + mi
                    for ci, (c0, csz) in enumerate(chunks):
                        nc.scalar.activation(
                            p[:, m * n + c0 : m * n + c0 + csz],
                            psbs[(mi, ci)][:],
                            AF.Exp,
                            bias=b2t[:, m : m + 1],
                        )

            ab_pools2.__exit__(None, None, None)
            ab_pools.__exit__(None, None, None)
            pss_ctx = tc.tile_pool(name="pss", bufs=1, space="PSUM")
            pss_pool = pss_ctx.__enter__()

            # ---- Phase C: normalize and scale by gate coefficient ----
            s_sb = gp.tile([1, n], f32)
            for ci, (c0, csz) in enumerate(chunks):
                sp = pss_pool.tile([1, csz], f32, tag="sp", name=f"sp_{ci}")
                for k in range(KD):
                    nc.tensor.matmul(
                        sp[:],
                        ones_col[:],
                        p[:, k * n + c0 : k * n + c0 + csz],
                        start=(k == 0),
                        stop=(k == KD - 1),
                    )
                nc.vector.tensor_copy(s_sb[0:1, c0 : c0 + csz], sp[:])
            # lane-parallel g = c / s via DRAM round-trip reshape
            nc.sync.dma_start(sdr[:].rearrange("(o f) -> o f", o=1), s_sb[0:1, :])
            slanes = gp.tile([P, n // P], f32)
            nc.sync.dma_start(slanes[:], sdr[:].rearrange("(p f) -> p f", p=P))
            rlanes = gp.tile([P, n // P], f32)
            nc.vector.reciprocal(rlanes[:], slanes[:])
            glanes = gp.tile([P, n // P], f32)
            nc.vector.tensor_mul(glanes[:], rlanes[:], clanes[:])
            nc.sync.dma_start(gdr[:].rearrange("(p f) -> p f", p=P), glanes[:])
            g_sb = gp.tile([1, n], f32)
            nc.sync.dma_start(g_sb[:], gdr[:].rearrange("(o f) -> o f", o=1))
            for ci, (c0, csz) in enumerate(chunks):
                gb_ps = pss_pool.tile([P, csz], f32, tag="gb", name=f"gb_{ci}")
                nc.tensor.matmul(
                    gb_ps[:],
                    ones_row[:],
                    g_sb[0:1, c0 : c0 + csz],
                    start=True,
                    stop=True,
                )
                gb = gp.tile([P, csz], f32, tag="gbs", name=f"gbs_{ci}")
                nc.vector.tensor_copy(gb[:], gb_ps[:])
                for k in range(KD):
                    nc.vector.tensor_mul(
                        p[:, k * n + c0 : k * n + c0 + csz],
                        p[:, k * n + c0 : k * n + c0 + csz],
                        gb[:],
                    )
            for k in range(KD):
                nc.sync.dma_start(outT[k * P : (k + 1) * P, :], p[:, k * n : (k + 1) * n])
            pss_ctx.__exit__(None, None, None)

    nc.finalize()
    return nc


def _get_nc(n, use_bf16):
    key = (n, use_bf16)
    if key not in _CACHE:
        _CACHE[key] = _build_nc(n, use_bf16)
    return _CACHE[key]


def kernel(x, gate_w, gate_b, w1, b1, w2, b2, top_k, use_bf16=False,
           _trace=False, _tmpdir=None):
    x = np.asarray(x)
    B, S, _ = x.shape
    T = B * S
    assert int(top_k) == 2
    c = _gating_coeffs(x, gate_w, gate_b)

    x_f = np.ascontiguousarray(x.reshape(T, D).astype(np.float32))
    idx = [np.nonzero(c[:, ex])[0] for ex in (0, 1)]  # tokens per expert
    per_core = max(
        (len(idx[0]) + 3) // 4, (len(idx[1]) + 3) // 4, 1
    )
    n = ((per_core + P - 1) // P) * P  # padded tokens per core

    if use_bf16:
        import ml_dtypes

        def conv(a):
            return np.ascontiguousarray(a.astype(ml_dtypes.bfloat16))
    else:
        conv = _round_f32r

    w1 = np.asarray(w1, dtype=np.float32)
    w2 = np.asarray(w2, dtype=np.float32)
    b1 = np.asarray(b1, dtype=np.float32)
    b2 = np.asarray(b2, dtype=np.float32)

    in_maps = []
    core_tok = []  # per-core real token ids
    for core in range(NCORES):
        ex = core // 4
        part = core % 4
        ids = idx[ex][part * per_core : (part + 1) * per_core]
        core_tok.append(ids)
        xTc = np.zeros((D, n), dtype=np.float32)
        if len(ids):
            xTc[:, : len(ids)] = x_f[ids].T
        cl = np.zeros(n, dtype=np.float32)
        cl[: len(ids)] = c[ids, ex]
        in_maps.append(
            {
                "xT": conv(xTc),
                "w1": conv(w1[ex]),
                "w2": conv(w2[ex]),
                "b1t": np.ascontiguousarray(b1[ex].reshape(F // P, P).T.astype(np.float32)),
                "b2t": np.ascontiguousarray(b2[ex].reshape(D // P, P).T.astype(np.float32)),
                "clanes": np.ascontiguousarray(cl.reshape(P, n // P)),
            }
        )

    nc = _get_nc(n, use_bf16)
    kw = {}
    if _trace:
        kw = {"trace": True, "tmpdir": _tmpdir}
    res = run_bass_kernel_spmd(nc, in_maps, core_ids=list(range(NCORES)), **kw)
    kernel.last_results = res

    out = np.zeros((T, D), dtype=np.float32)
    for core in range(NCORES):
        ids = core_tok[core]
        if len(ids) == 0:
            continue
        contrib = res.results[core]["outT"][:, : len(ids)].T  # [n_real, D]
        out[ids] += contrib
    return out.reshape(B, S, D)


kernel.last_results = None
